# revision 17
# baseline (speedup 1.0000x reference)
"""Trainium2 Bass kernel for nn_ComplexConv2Deffangle4Dxy.

Reference math (per batch b, branch br):
    out[br] = pointwise(w2, depthwise3x3(w1, img[br]))   with zero padding P=1
      br=0 (rot): weights (w1n, w2n) where wn = (wx+wy)^2 / sum((wx+wy)^2)
      br=1 (abs): log-domain: exp(branch(log(img + EPS), w1n, w2n))
      br=2 (x):   weights (w1x, w2x)
      br=3 (y):   weights (w1y, w2y)

Kernel strategy (per NeuronCore, data-parallel over batch B=8 -> 8 cores):
  Fuse depthwise+pointwise into a single 3x3 conv whose weights are the
  outer product  Wf[o, c, k] = w2[o, c] * w1[c, k], computed as
  PSUM-accumulated matmuls over the 9 kernel offsets with
  lhsT = fused weights (K=Cin=64, M=Cout=128) and rhs = shifted image views.
  Images are zero-padded on the host (pure marshaling); for the abs branch
  Ln(x+EPS) maps the zero padding to log(EPS), matching the reference's
  pad-then-log order.  Weight normalization for the rot/abs branches is
  computed on device.

  Image layout ("hsplit"): SBUF partitions 0..63 hold padded rows 0..34,
  partitions 64..127 hold padded rows 31..65.  Output row-tiles 0..3 read
  the lower half (PE row groups 0-1), tiles 4..7 the upper half (row
  groups 2-3).

  Matmul issue order: tap-outer, tile-inner, alternating lower/upper so
  consecutive matmuls land on disjoint PE row groups and overlap in the
  array; all 8 PSUM banks hold live accumulators so weights stay loaded
  across the 4 tiles of each (tap, half).  bf16 operands enable split
  LDWEIGHTS (walrus --enable-ldw-opt) + fast weight load; outputs are
  DMA'd as bf16 and upcast on host (all within the rel-err budget).
"""

import sys

for _p in ("/opt/trn_rl_repo",):
    if _p not in sys.path:
        sys.path.insert(0, _p)

import ml_dtypes
import numpy as np

import concourse.bacc as bacc
import concourse.mybir as mybir
import concourse.tile as tile
from concourse import bass_utils

F32 = mybir.dt.float32
F32R = mybir.dt.float32r
BF16 = mybir.dt.bfloat16

EPS = 1e-6
N_CORES = 8
B, NBR, CIN, COUT, H, W = 8, 4, 64, 128, 64, 64
HP, WP = H + 2, W + 2          # host-padded image
HS_ROWS = 35                   # hsplit: padded rows per partition half

MM_DTYPE = "bf16"              # matmul input dtype: "f32r" | "f32" | "bf16"
OUT_BF16 = True                # DMA outputs as bf16, upcast to f32 on host
ISSUE = "hb"                   # "hb" (half-branch pipelined) | "ilv" | "seq"
DEDUPE_LDW = True              # drop redundant same-row-group weight reloads
LOOP_ITERS = None              # benchmarking: device-side repeat count
PROBE = ""                     # "" | "no_out" (skip evac+out-DMA) | "no_mm"
TRACE = False
LAST_EXEC_TIME_NS = None
LAST_RESULTS = None

_PROG_CACHE = {}

# walrus's LDWEIGHTS optimization: NOT needed — bass already splits bf16
# matmuls into InstLdweights + InstMatmult at the BIR level (which is what
# lets the PE reorder window pull weight loads ahead).  Enabling the walrus
# flag on the pre-split form fails codegen ("InstLdweights is not compatible
# with LDW optimization"), so keep False.
LDW_OPT = False
_orig_run_command = bass_utils.run_command


def _patched_run_command(cmd, *a, **kw):
    if (
        LDW_OPT
        and MM_DTYPE == "bf16"
        and isinstance(cmd, list)
        and "--enable-ldw-opt=false" in cmd
    ):
        cmd = ["--enable-ldw-opt=true" if c == "--enable-ldw-opt=false" else c for c in cmd]
    return _orig_run_command(cmd, *a, **kw)


bass_utils.run_command = _patched_run_command
if getattr(bass_utils, "bir_verify_and_optimise", None) is not None:
    bass_utils.bir_verify_and_optimise.__globals__["run_command"] = _patched_run_command

BRANCHES = (  # (branch index, weight set, log-domain?)
    (2, "x", False),
    (3, "y", False),
    (0, "n", False),
    (1, "n", True),
)


def _mm_dt():
    return {"f32r": F32R, "f32": F32, "bf16": BF16}[MM_DTYPE]


def _np_in_dt():
    return ml_dtypes.bfloat16 if MM_DTYPE == "bf16" else np.float32


def _out_dt():
    return BF16 if OUT_BF16 else F32


def _emit(nc, tc, xin_d, w1x_d, w1y_d, w2xT_d, w2yT_d, out_d):
    mdt = _mm_dt()
    odt = _out_dt()
    with (
        tc.tile_pool(name="wp", bufs=1) as wp,
        tc.tile_pool(name="imgp", bufs=2) as imgp,
        tc.tile_pool(name="psp", bufs=8, space="PSUM") as psp,
        tc.tile_pool(name="obp", bufs=8) as obp,
    ):
        # ---- weight prep -------------------------------------------------
        # All weight/source tiles replicated into both partition halves so
        # both PE row groups see the same fused weights.
        w1x_s = wp.tile([2 * CIN, 9], F32, tag="w1x")
        w1y_s = wp.tile([2 * CIN, 9], F32, tag="w1y")
        w2xT_s = wp.tile([2 * CIN, COUT], F32, tag="w2xT")
        w2yT_s = wp.tile([2 * CIN, COUT], F32, tag="w2yT")
        for t, d in (
            (w1x_s, w1x_d),
            (w1y_s, w1y_d),
            (w2xT_s, w2xT_d),
            (w2yT_s, w2yT_d),
        ):
            nc.sync.dma_start(out=t[0:CIN], in_=d)
            nc.sync.dma_start(out=t[CIN : 2 * CIN], in_=d)

        ones_k = wp.tile([CIN, 1], F32, tag="ones_k")
        nc.vector.memset(ones_k[:, :], 1.0)
        ones_m = wp.tile([1, 2 * CIN], F32, tag="ones_m")
        nc.vector.memset(ones_m[:, :], 1.0)
        eps_b = wp.tile([2 * CIN, 1], F32, tag="eps_b")
        nc.vector.memset(eps_b[:, :], float(EPS))
        zero_b = wp.tile([COUT, 1], F32, tag="zero_b")
        nc.vector.memset(zero_b[:, :], 0.0)

        # u1 = (w1x + w1y)^2, u2T = ((w2x + w2y)^2)^T  (both partition halves)
        u1 = wp.tile([2 * CIN, 9], F32, tag="u1")
        nc.vector.tensor_add(u1[:, :], w1x_s[:, :], w1y_s[:, :])
        nc.vector.tensor_mul(u1[:, :], u1[:, :], u1[:, :])
        u2T = wp.tile([2 * CIN, COUT], F32, tag="u2T")
        nc.vector.tensor_add(u2T[:, :], w2xT_s[:, :], w2yT_s[:, :])
        nc.vector.tensor_mul(u2T[:, :], u2T[:, :], u2T[:, :])

        # S1 = sum(u1), S2 = sum(u2) via ones-matmul + free-dim reduce
        s1v = psp.tile([1, 9], F32, tag="ps")
        nc.tensor.matmul(s1v[:, :], ones_k[:, :], u1[0:CIN, :], start=True, stop=True)
        s2v = psp.tile([1, COUT], F32, tag="ps")
        nc.tensor.matmul(s2v[:, :], ones_k[:, :], u2T[0:CIN, :], start=True, stop=True)
        s1 = wp.tile([1, 1], F32, tag="s1")
        nc.vector.tensor_reduce(
            s1[:, :], s1v[:, :], axis=mybir.AxisListType.X, op=mybir.AluOpType.add
        )
        s2 = wp.tile([1, 1], F32, tag="s2")
        nc.vector.tensor_reduce(
            s2[:, :], s2v[:, :], axis=mybir.AxisListType.X, op=mybir.AluOpType.add
        )
        inv = wp.tile([1, 1], F32, tag="inv")
        nc.vector.tensor_mul(inv[:, :], s1[:, :], s2[:, :])
        nc.vector.reciprocal(inv[:, :], inv[:, :])
        # broadcast 1/(S1*S2) to all 128 partitions
        invb_ps = psp.tile([2 * CIN, 1], F32, tag="ps")
        nc.tensor.matmul(invb_ps[:, :], ones_m[:, :], inv[:, :], start=True, stop=True)
        invb = wp.tile([2 * CIN, 1], F32, tag="invb")
        nc.vector.tensor_copy(invb[:, :], invb_ps[:, :])
        # u2T_n = u2T / (S1*S2): both normalizations in one fold
        u2Tn = wp.tile([2 * CIN, COUT], F32, tag="u2Tn")
        nc.vector.tensor_scalar(
            u2Tn[:, :], u2T[:, :], invb[:, 0:1], None, mybir.AluOpType.mult
        )

        # fused weight tiles: 9 column blocks, block k = w2T * w1[:, k],
        # identical in both partition halves (built in one op across 128
        # partitions since the scalar operand is partition-local).
        wf_tiles = {}
        for s, base, w1s in (("x", w2xT_s, w1x_s), ("y", w2yT_s, w1y_s), ("n", u2Tn, u1)):
            wf = wp.tile([2 * CIN, 9 * COUT], mdt, tag=f"wf{s}")
            for k in range(9):
                nc.vector.tensor_scalar(
                    wf[:, k * COUT : (k + 1) * COUT],
                    base[:, :],
                    w1s[:, k : k + 1],
                    None,
                    mybir.AluOpType.mult,
                )
            wf_tiles[s] = wf

        # ---- main compute ------------------------------------------------
        def emit_evac2(b, needs_log, ps2, h0, dve):
            """Evacuate 2 PSUM tiles (16 out rows) into one SBUF buffer,
            single 256KB DMA out."""
            ot = obp.tile([COUT, 2, 8, W], odt, tag="ot")
            for i in range(2):
                if needs_log:
                    nc.scalar.activation(
                        ot[:, i],
                        ps2[i][:, :, :],
                        mybir.ActivationFunctionType.Exp,
                        bias=zero_b[:, 0:1],
                    )
                elif dve:
                    nc.vector.tensor_copy(ot[:, i], ps2[i][:, :, :])
                else:
                    nc.scalar.activation(
                        ot[:, i], ps2[i][:, :, :], mybir.ActivationFunctionType.Copy
                    )
            nc.sync.dma_start(out=out_d[b, :, h0 : h0 + 16, :], in_=ot[:, :, :, :])

        def main_body():
            for b, s, needs_log in BRANCHES:
                wf = wf_tiles[s]
                img = imgp.tile([2 * CIN, HS_ROWS, WP], mdt, tag="img")
                nc.sync.dma_start(out=img[:, :, :], in_=xin_d[b])
                if needs_log:
                    nc.scalar.activation(
                        img[:, :, :],
                        img[:, :, :],
                        mybir.ActivationFunctionType.Ln,
                        bias=eps_b[:, 0:1],
                    )
                if ISSUE == "hb":
                    # Two half-branches, each 2 lower + 2 upper row-tiles
                    # (both PE row groups stay busy); the first half-branch's
                    # evac + output DMA overlap the second's matmuls.
                    for hb in range(2):
                        ps4 = [
                            psp.tile([COUT, 8, W], F32, tag="ps", name=f"ps{t}")
                            for t in range(4)
                        ]
                        if PROBE != "no_mm":
                            _mm_hb(nc, ps4, wf, img, hb)
                        if PROBE != "no_out":
                            # ps4[0:2] = lower tiles 2hb,2hb+1 -> rows 16hb..
                            # ps4[2:4] = upper tiles -> rows 32+16hb..
                            emit_evac2(b, needs_log, ps4[0:2], 16 * hb, dve=True)
                            emit_evac2(
                                b, needs_log, ps4[2:4], 32 + 16 * hb, dve=False
                            )
                    continue
                ps = [
                    psp.tile([COUT, 8, W], F32, tag="ps", name=f"ps{t}")
                    for t in range(8)
                ]
                if PROBE != "no_mm":
                    if ISSUE == "ilv":
                        _mm_ilv(nc, ps, wf, img)
                    else:
                        _mm_seq(nc, ps, wf, img)
                if PROBE == "no_out":
                    continue
                for i in range(4):
                    emit_evac2(
                        b,
                        needs_log,
                        [ps[2 * i], ps[2 * i + 1]],
                        16 * (i % 2) + 32 * (i // 2),
                        dve=(i % 2 == 0),
                    )

        if LOOP_ITERS:
            with tc.For_i(0, LOOP_ITERS, 1):
                main_body()
        else:
            main_body()


def _rhs(img, half, tpl, k):
    """Shifted image view for out-row-tile (half, tpl) and tap k."""
    dh, dw = k // 3 - 1, k % 3 - 1
    p0, p1 = half * CIN, (half + 1) * CIN
    r = 8 * tpl + 1 + dh + half  # lower: pad row - 0; upper: pad row - 31
    c0 = 1 + dw
    return img[p0:p1, r : r + 8, c0 : c0 + W]


def _wfk(wf, k, half):
    p0, p1 = half * CIN, (half + 1) * CIN
    return wf[p0:p1, k * COUT : (k + 1) * COUT]


def _mm_ilv(nc, ps, wf, img):
    """Tap-outer, tile-inner, alternating lower/upper row groups.

    Consecutive matmuls target disjoint PE row groups (tile_position derives
    from lhsT base_partition: 0 vs 64) and distinct PSUM banks, so they run
    concurrently in the array; within a (tap, half) the 4 tiles share one
    weight load."""
    for k in range(9):
        st, sp = k == 0, k == 8
        for tpl in range(4):
            nc.tensor.matmul(
                ps[tpl][:, :, :],
                _wfk(wf, k, 0),
                _rhs(img, 0, tpl, k),
                start=st,
                stop=sp,
                skip_group_check=True,
            )
            nc.tensor.matmul(
                ps[tpl + 4][:, :, :],
                _wfk(wf, k, 1),
                _rhs(img, 1, tpl, k),
                start=st,
                stop=sp,
                skip_group_check=True,
            )


def _mm_hb(nc, ps4, wf, img, hb):
    """Half-branch hb: lower tiles {2hb, 2hb+1} -> ps4[0:2], upper tiles
    {2hb, 2hb+1} -> ps4[2:4].  Tap-outer so each (tap, half) shares one
    weight load across 2 tiles; lower/upper alternate for row-group
    concurrency.  NOTE: concurrent K=32 same-bank accumulation (4-way row
    tiling) crashes TRN2 at runtime ("mesh desynced") -- don't."""
    for k in range(9):
        st, sp = k == 0, k == 8
        for j in range(2):
            tpl = 2 * hb + j
            nc.tensor.matmul(
                ps4[j][:, :, :],
                _wfk(wf, k, 0),
                _rhs(img, 0, tpl, k),
                start=st,
                stop=sp,
                skip_group_check=True,
            )
            nc.tensor.matmul(
                ps4[2 + j][:, :, :],
                _wfk(wf, k, 1),
                _rhs(img, 1, tpl, k),
                start=st,
                stop=sp,
                skip_group_check=True,
            )


def _mm_seq(nc, ps, wf, img):
    """Baseline order: tile-outer, tap-inner (each tile's 9 taps serial)."""
    for t in range(8):
        half, tpl = (0, t) if t < 4 else (1, t - 4)
        for k in range(9):
            nc.tensor.matmul(
                ps[t][:, :, :],
                _wfk(wf, k, half),
                _rhs(img, half, tpl, k),
                start=(k == 0),
                stop=(k == 8),
                skip_group_check=True,
            )


def _ldw_key(inst):
    ap = inst.ins[0]
    bap = ap.bass_ap
    if bap is None:
        return None
    return (
        bap.tensor.name,
        bap.offset,
        tuple(tuple(p) for p in bap.ap),
        inst.tile_position,
        inst.tile_size,
        getattr(inst, "perf_mode", None),
        getattr(inst, "is_transpose", None),
    )


def dedupe_ldweights(nc):
    """Drop InstLdweights that reload the exact weights already resident in
    the same PE row group (bass emits one load per matmul even when
    consecutive same-row-group matmuls share lhsT).  Redundant loads cost
    ~53ns of serial PE time each since a row group's load cannot overlap its
    own in-flight matmul.  Only syncless duplicates are dropped, and tracking
    resets at every block boundary and at any PE instruction that could
    disturb the array (transpose-mode load, non-matmul PE op)."""
    dropped = 0
    for fn in nc.m.functions:
        for blk in fn.blocks:
            resident = {}  # tile_position[0] (row group base) -> ldw key
            keep = []
            for inst in blk.instructions:
                n = type(inst).__name__
                if n == "InstLdweights":
                    si = inst.sync_info
                    clean = si is None or (not si.on_wait and not si.on_update)
                    key = _ldw_key(inst)
                    if inst.is_transpose:
                        resident.clear()
                    elif (
                        clean
                        and key is not None
                        and inst.tile_position is not None
                        and resident.get(inst.tile_position[0]) == key
                    ):
                        dropped += 1
                        continue  # redundant: same weights already loaded
                    elif key is not None and inst.tile_position is not None:
                        resident[inst.tile_position[0]] = key
                    else:
                        resident.clear()
                elif n == "InstMatmult":
                    if inst.is_transpose:
                        resident.clear()
                else:
                    pass  # non-PE instructions don't touch the array
                keep.append(inst)
            blk.instructions[:] = keep
    return dropped


def build_program():
    key = (MM_DTYPE, ISSUE, OUT_BF16, LOOP_ITERS, PROBE, LDW_OPT, DEDUPE_LDW)
    if key in _PROG_CACHE:
        return _PROG_CACHE[key]
    nc = bacc.Bacc("TRN2", target_bir_lowering=False, debug=False)
    xin_d = nc.dram_tensor(
        "xin", [NBR, 2 * CIN, HS_ROWS, WP], _mm_dt(), kind="ExternalInput"
    ).ap()
    w1x_d = nc.dram_tensor("w1x", [CIN, 9], F32, kind="ExternalInput").ap()
    w1y_d = nc.dram_tensor("w1y", [CIN, 9], F32, kind="ExternalInput").ap()
    w2xT_d = nc.dram_tensor("w2xT", [CIN, COUT], F32, kind="ExternalInput").ap()
    w2yT_d = nc.dram_tensor("w2yT", [CIN, COUT], F32, kind="ExternalInput").ap()
    out_d = nc.dram_tensor("out", [NBR, COUT, H, W], _out_dt(), kind="ExternalOutput").ap()
    with tile.TileContext(nc) as tc:
        _emit(nc, tc, xin_d, w1x_d, w1y_d, w2xT_d, w2yT_d, out_d)
    nc.compile()
    if MM_DTYPE == "bf16" and DEDUPE_LDW:
        dedupe_ldweights(nc)
    _PROG_CACHE[key] = nc
    return nc


def marshal_inputs(x, w1x, w1y, w2x, w2y):
    """Host-side data marshaling: shard over batch, zero-pad, build the
    per-partition-half copies (hsplit layout)."""
    ndt = _np_in_dt()
    x = np.asarray(x, dtype=np.float32)
    xp = np.zeros((B, NBR, CIN, HP, WP), np.float32)
    xp[:, :, :, 1 : H + 1, 1 : W + 1] = x
    xin = np.empty((B, NBR, 2 * CIN, HS_ROWS, WP), ndt)
    xin[:, :, 0:CIN] = xp[:, :, :, 0:HS_ROWS, :].astype(ndt)
    xin[:, :, CIN:] = xp[:, :, :, HP - HS_ROWS : HP, :].astype(ndt)
    w2xT = np.ascontiguousarray(np.asarray(w2x, np.float32).T)
    w2yT = np.ascontiguousarray(np.asarray(w2y, np.float32).T)
    w1x = np.ascontiguousarray(w1x, np.float32)
    w1y = np.ascontiguousarray(w1y, np.float32)
    return [
        {
            "xin": np.ascontiguousarray(xin[i]),
            "w1x": w1x,
            "w1y": w1y,
            "w2xT": w2xT,
            "w2yT": w2yT,
        }
        for i in range(B)
    ]


def kernel(x, w1x, w1y, w2x, w2y):
    global LAST_EXEC_TIME_NS, LAST_RESULTS
    nc = build_program()
    in_maps = marshal_inputs(x, w1x, w1y, w2x, w2y)
    res = bass_utils.run_bass_kernel_spmd(
        nc, in_maps, list(range(N_CORES)), trace=TRACE
    )
    LAST_EXEC_TIME_NS = res.exec_time_ns
    LAST_RESULTS = res
    out = np.stack([res.results[i]["out"] for i in range(N_CORES)], axis=0)
    return np.asarray(out, np.float32)


# revision 19
# speedup vs baseline: 2.4747x; 2.4747x over previous
"""Trainium2 Bass kernel for nn_ComplexConv2Deffangle4Dxy.

Reference math (per batch b, branch br):
    out[br] = pointwise(w2, depthwise3x3(w1, img[br]))   with zero padding P=1
      br=0 (rot): weights (w1n, w2n) where wn = (wx+wy)^2 / sum((wx+wy)^2)
      br=1 (abs): log-domain: exp(branch(log(img + EPS), w1n, w2n))
      br=2 (x):   weights (w1x, w2x)
      br=3 (y):   weights (w1y, w2y)

Kernel strategy (per NeuronCore, data-parallel over batch B=8 -> 8 cores):
  Fuse depthwise+pointwise into a single 3x3 conv whose weights are the
  outer product  Wf[o, c, k] = w2[o, c] * w1[c, k], computed as
  PSUM-accumulated matmuls over the 9 kernel offsets with
  lhsT = fused weights (K=Cin=64, M=Cout=128) and rhs = shifted image views.
  Images are zero-padded on the host (pure marshaling); for the abs branch
  Ln(x+EPS) maps the zero padding to log(EPS), matching the reference's
  pad-then-log order.  Weight normalization for the rot/abs branches is
  computed on device.

  Image layout ("hsplit"): SBUF partitions 0..63 hold padded rows 0..34,
  partitions 64..127 hold padded rows 31..65.  Output row-tiles 0..3 read
  the lower half (PE row groups 0-1), tiles 4..7 the upper half (row
  groups 2-3).

  Matmul issue order: tap-outer, tile-inner, alternating lower/upper so
  consecutive matmuls land on disjoint PE row groups and overlap in the
  array; all 8 PSUM banks hold live accumulators so weights stay loaded
  across the 4 tiles of each (tap, half).  bf16 operands enable split
  LDWEIGHTS (walrus --enable-ldw-opt) + fast weight load; outputs are
  DMA'd as bf16 and upcast on host (all within the rel-err budget).
"""

import sys

for _p in ("/opt/trn_rl_repo",):
    if _p not in sys.path:
        sys.path.insert(0, _p)

import ml_dtypes
import numpy as np

import concourse.bacc as bacc
import concourse.mybir as mybir
import concourse.tile as tile
from concourse import bass_utils

F32 = mybir.dt.float32
F32R = mybir.dt.float32r
BF16 = mybir.dt.bfloat16

EPS = 1e-6
N_CORES = 8
B, NBR, CIN, COUT, H, W = 8, 4, 64, 128, 64, 64
HP, WP = H + 2, W + 2          # host-padded image
HS_ROWS = 35                   # hsplit: padded rows per partition half

MM_DTYPE = "bf16"              # matmul input dtype: "f32r" | "f32" | "bf16"
OUT_BF16 = True                # DMA outputs as bf16, upcast to f32 on host
ISSUE = "hb"                   # "hb" (half-branch pipelined) | "ilv" | "seq"
# Dropping redundant same-row-group weight reloads REGRESSES on HW (53.3us
# vs 49.3us): walrus pairs each InstLdweights with its InstMatmult, and an
# unpaired matmul lowers to a slower self-loading form.  Keep False.
DEDUPE_LDW = False
LOOP_ITERS = None              # benchmarking: device-side repeat count
PROBE = ""                     # "" | "no_out" (skip evac+out-DMA) | "no_mm"
TRACE = False
LAST_EXEC_TIME_NS = None
LAST_RESULTS = None

_PROG_CACHE = {}

# walrus's LDWEIGHTS optimization: NOT needed — bass already splits bf16
# matmuls into InstLdweights + InstMatmult at the BIR level (which is what
# lets the PE reorder window pull weight loads ahead).  Enabling the walrus
# flag on the pre-split form fails codegen ("InstLdweights is not compatible
# with LDW optimization"), so keep False.
LDW_OPT = False
_orig_run_command = bass_utils.run_command


def _patched_run_command(cmd, *a, **kw):
    if (
        LDW_OPT
        and MM_DTYPE == "bf16"
        and isinstance(cmd, list)
        and "--enable-ldw-opt=false" in cmd
    ):
        cmd = ["--enable-ldw-opt=true" if c == "--enable-ldw-opt=false" else c for c in cmd]
    return _orig_run_command(cmd, *a, **kw)


bass_utils.run_command = _patched_run_command
if getattr(bass_utils, "bir_verify_and_optimise", None) is not None:
    bass_utils.bir_verify_and_optimise.__globals__["run_command"] = _patched_run_command

BRANCHES = (  # (branch index, weight set, log-domain?)
    (2, "x", False),
    (3, "y", False),
    (0, "n", False),
    (1, "n", True),
)


def _mm_dt():
    return {"f32r": F32R, "f32": F32, "bf16": BF16}[MM_DTYPE]


def _np_in_dt():
    return ml_dtypes.bfloat16 if MM_DTYPE == "bf16" else np.float32


def _out_dt():
    return BF16 if OUT_BF16 else F32


def _emit(nc, tc, xin_d, w1x_d, w1y_d, w2xT_d, w2yT_d, out_d):
    mdt = _mm_dt()
    odt = _out_dt()
    with (
        tc.tile_pool(name="wp", bufs=1) as wp,
        tc.tile_pool(name="imgp", bufs=2) as imgp,
        tc.tile_pool(name="psp", bufs=8, space="PSUM") as psp,
        tc.tile_pool(name="obp", bufs=8) as obp,
    ):
        # ---- weight prep -------------------------------------------------
        # All weight/source tiles replicated into both partition halves so
        # both PE row groups see the same fused weights.
        w1x_s = wp.tile([2 * CIN, 9], F32, tag="w1x")
        w1y_s = wp.tile([2 * CIN, 9], F32, tag="w1y")
        w2xT_s = wp.tile([2 * CIN, COUT], F32, tag="w2xT")
        w2yT_s = wp.tile([2 * CIN, COUT], F32, tag="w2yT")
        for t, d in (
            (w1x_s, w1x_d),
            (w1y_s, w1y_d),
            (w2xT_s, w2xT_d),
            (w2yT_s, w2yT_d),
        ):
            nc.sync.dma_start(out=t[0:CIN], in_=d)
            nc.sync.dma_start(out=t[CIN : 2 * CIN], in_=d)

        ones_k = wp.tile([CIN, 1], F32, tag="ones_k")
        nc.vector.memset(ones_k[:, :], 1.0)
        ones_m = wp.tile([1, 2 * CIN], F32, tag="ones_m")
        nc.vector.memset(ones_m[:, :], 1.0)
        eps_b = wp.tile([2 * CIN, 1], F32, tag="eps_b")
        nc.vector.memset(eps_b[:, :], float(EPS))
        zero_b = wp.tile([COUT, 1], F32, tag="zero_b")
        nc.vector.memset(zero_b[:, :], 0.0)

        # u1 = (w1x + w1y)^2, u2T = ((w2x + w2y)^2)^T  (both partition halves)
        u1 = wp.tile([2 * CIN, 9], F32, tag="u1")
        nc.vector.tensor_add(u1[:, :], w1x_s[:, :], w1y_s[:, :])
        nc.vector.tensor_mul(u1[:, :], u1[:, :], u1[:, :])
        u2T = wp.tile([2 * CIN, COUT], F32, tag="u2T")
        nc.vector.tensor_add(u2T[:, :], w2xT_s[:, :], w2yT_s[:, :])
        nc.vector.tensor_mul(u2T[:, :], u2T[:, :], u2T[:, :])

        # S1 = sum(u1), S2 = sum(u2) via ones-matmul + free-dim reduce
        s1v = psp.tile([1, 9], F32, tag="ps")
        nc.tensor.matmul(s1v[:, :], ones_k[:, :], u1[0:CIN, :], start=True, stop=True)
        s2v = psp.tile([1, COUT], F32, tag="ps")
        nc.tensor.matmul(s2v[:, :], ones_k[:, :], u2T[0:CIN, :], start=True, stop=True)
        s1 = wp.tile([1, 1], F32, tag="s1")
        nc.vector.tensor_reduce(
            s1[:, :], s1v[:, :], axis=mybir.AxisListType.X, op=mybir.AluOpType.add
        )
        s2 = wp.tile([1, 1], F32, tag="s2")
        nc.vector.tensor_reduce(
            s2[:, :], s2v[:, :], axis=mybir.AxisListType.X, op=mybir.AluOpType.add
        )
        inv = wp.tile([1, 1], F32, tag="inv")
        nc.vector.tensor_mul(inv[:, :], s1[:, :], s2[:, :])
        nc.vector.reciprocal(inv[:, :], inv[:, :])
        # broadcast 1/(S1*S2) to all 128 partitions
        invb_ps = psp.tile([2 * CIN, 1], F32, tag="ps")
        nc.tensor.matmul(invb_ps[:, :], ones_m[:, :], inv[:, :], start=True, stop=True)
        invb = wp.tile([2 * CIN, 1], F32, tag="invb")
        nc.vector.tensor_copy(invb[:, :], invb_ps[:, :])
        # u2T_n = u2T / (S1*S2): both normalizations in one fold
        u2Tn = wp.tile([2 * CIN, COUT], F32, tag="u2Tn")
        nc.vector.tensor_scalar(
            u2Tn[:, :], u2T[:, :], invb[:, 0:1], None, mybir.AluOpType.mult
        )

        # fused weight tiles: 9 column blocks, block k = w2T * w1[:, k],
        # identical in both partition halves (built in one op across 128
        # partitions since the scalar operand is partition-local).
        wf_tiles = {}
        for s, base, w1s in (("x", w2xT_s, w1x_s), ("y", w2yT_s, w1y_s), ("n", u2Tn, u1)):
            wf = wp.tile([2 * CIN, 9 * COUT], mdt, tag=f"wf{s}")
            for k in range(9):
                nc.vector.tensor_scalar(
                    wf[:, k * COUT : (k + 1) * COUT],
                    base[:, :],
                    w1s[:, k : k + 1],
                    None,
                    mybir.AluOpType.mult,
                )
            wf_tiles[s] = wf

        # ---- main compute ------------------------------------------------
        def emit_evac2(b, needs_log, ps2, h0, dve):
            """Evacuate 2 PSUM tiles (16 out rows) into one SBUF buffer,
            single 256KB DMA out."""
            ot = obp.tile([COUT, 2, 8, W], odt, tag="ot")
            for i in range(2):
                if needs_log:
                    nc.scalar.activation(
                        ot[:, i],
                        ps2[i][:, :, :],
                        mybir.ActivationFunctionType.Exp,
                        bias=zero_b[:, 0:1],
                    )
                elif dve:
                    nc.vector.tensor_copy(ot[:, i], ps2[i][:, :, :])
                else:
                    nc.scalar.activation(
                        ot[:, i], ps2[i][:, :, :], mybir.ActivationFunctionType.Copy
                    )
            nc.sync.dma_start(out=out_d[b, :, h0 : h0 + 16, :], in_=ot[:, :, :, :])

        def main_body():
            for b, s, needs_log in BRANCHES:
                wf = wf_tiles[s]
                img = imgp.tile([2 * CIN, HS_ROWS, WP], mdt, tag="img")
                nc.sync.dma_start(out=img[:, :, :], in_=xin_d[b])
                if needs_log:
                    nc.scalar.activation(
                        img[:, :, :],
                        img[:, :, :],
                        mybir.ActivationFunctionType.Ln,
                        bias=eps_b[:, 0:1],
                    )
                if ISSUE == "hb":
                    # Two half-branches, each 2 lower + 2 upper row-tiles
                    # (both PE row groups stay busy); the first half-branch's
                    # evac + output DMA overlap the second's matmuls.
                    for hb in range(2):
                        ps4 = [
                            psp.tile([COUT, 8, W], F32, tag="ps", name=f"ps{t}")
                            for t in range(4)
                        ]
                        if PROBE != "no_mm":
                            _mm_hb(nc, ps4, wf, img, hb)
                        if PROBE != "no_out":
                            # ps4[0:2] = lower tiles 2hb,2hb+1 -> rows 16hb..
                            # ps4[2:4] = upper tiles -> rows 32+16hb..
                            # All non-log evacs on DVE: keeps ACT free for
                            # the abs branch's Ln + Exp.
                            emit_evac2(b, needs_log, ps4[0:2], 16 * hb, dve=True)
                            emit_evac2(
                                b, needs_log, ps4[2:4], 32 + 16 * hb, dve=True
                            )
                    continue
                ps = [
                    psp.tile([COUT, 8, W], F32, tag="ps", name=f"ps{t}")
                    for t in range(8)
                ]
                if PROBE != "no_mm":
                    if ISSUE == "ilv":
                        _mm_ilv(nc, ps, wf, img)
                    else:
                        _mm_seq(nc, ps, wf, img)
                if PROBE == "no_out":
                    continue
                for i in range(4):
                    emit_evac2(
                        b,
                        needs_log,
                        [ps[2 * i], ps[2 * i + 1]],
                        16 * (i % 2) + 32 * (i // 2),
                        dve=(i % 2 == 0),
                    )

        if LOOP_ITERS:
            with tc.For_i(0, LOOP_ITERS, 1):
                main_body()
        else:
            main_body()


def _rhs(img, half, tpl, k):
    """Shifted image view for out-row-tile (half, tpl) and tap k."""
    dh, dw = k // 3 - 1, k % 3 - 1
    p0, p1 = half * CIN, (half + 1) * CIN
    r = 8 * tpl + 1 + dh + half  # lower: pad row - 0; upper: pad row - 31
    c0 = 1 + dw
    return img[p0:p1, r : r + 8, c0 : c0 + W]


def _wfk(wf, k, half):
    p0, p1 = half * CIN, (half + 1) * CIN
    return wf[p0:p1, k * COUT : (k + 1) * COUT]


def _mm_ilv(nc, ps, wf, img):
    """Tap-outer, tile-inner, alternating lower/upper row groups.

    Consecutive matmuls target disjoint PE row groups (tile_position derives
    from lhsT base_partition: 0 vs 64) and distinct PSUM banks, so they run
    concurrently in the array; within a (tap, half) the 4 tiles share one
    weight load."""
    for k in range(9):
        st, sp = k == 0, k == 8
        for tpl in range(4):
            nc.tensor.matmul(
                ps[tpl][:, :, :],
                _wfk(wf, k, 0),
                _rhs(img, 0, tpl, k),
                start=st,
                stop=sp,
                skip_group_check=True,
            )
            nc.tensor.matmul(
                ps[tpl + 4][:, :, :],
                _wfk(wf, k, 1),
                _rhs(img, 1, tpl, k),
                start=st,
                stop=sp,
                skip_group_check=True,
            )


def _mm_hb(nc, ps4, wf, img, hb):
    """Half-branch hb: lower tiles {2hb, 2hb+1} -> ps4[0:2], upper tiles
    {2hb, 2hb+1} -> ps4[2:4].  Tap-outer so each (tap, half) shares one
    weight load across 2 tiles; lower/upper alternate for row-group
    concurrency.  NOTE: concurrent K=32 same-bank accumulation (4-way row
    tiling) crashes TRN2 at runtime ("mesh desynced") -- don't."""
    for k in range(9):
        st, sp = k == 0, k == 8
        for j in range(2):
            tpl = 2 * hb + j
            nc.tensor.matmul(
                ps4[j][:, :, :],
                _wfk(wf, k, 0),
                _rhs(img, 0, tpl, k),
                start=st,
                stop=sp,
                skip_group_check=True,
            )
            nc.tensor.matmul(
                ps4[2 + j][:, :, :],
                _wfk(wf, k, 1),
                _rhs(img, 1, tpl, k),
                start=st,
                stop=sp,
                skip_group_check=True,
            )


def _mm_seq(nc, ps, wf, img):
    """Baseline order: tile-outer, tap-inner (each tile's 9 taps serial)."""
    for t in range(8):
        half, tpl = (0, t) if t < 4 else (1, t - 4)
        for k in range(9):
            nc.tensor.matmul(
                ps[t][:, :, :],
                _wfk(wf, k, half),
                _rhs(img, half, tpl, k),
                start=(k == 0),
                stop=(k == 8),
                skip_group_check=True,
            )


def _ldw_key(inst):
    ap = inst.ins[0]
    bap = ap.bass_ap
    if bap is None:
        return None
    return (
        bap.tensor.name,
        bap.offset,
        tuple(tuple(p) for p in bap.ap),
        inst.tile_position,
        inst.tile_size,
        getattr(inst, "perf_mode", None),
        getattr(inst, "is_transpose", None),
    )


def dedupe_ldweights(nc):
    """Drop InstLdweights that reload the exact weights already resident in
    the same PE row group (bass emits one load per matmul even when
    consecutive same-row-group matmuls share lhsT).  Redundant loads cost
    ~53ns of serial PE time each since a row group's load cannot overlap its
    own in-flight matmul.  Only syncless duplicates are dropped, and tracking
    resets at every block boundary and at any PE instruction that could
    disturb the array (transpose-mode load, non-matmul PE op)."""
    dropped = 0
    for fn in nc.m.functions:
        for blk in fn.blocks:
            resident = {}  # tile_position[0] (row group base) -> ldw key
            keep = []
            for inst in blk.instructions:
                n = type(inst).__name__
                if n == "InstLdweights":
                    si = inst.sync_info
                    clean = si is None or (not si.on_wait and not si.on_update)
                    key = _ldw_key(inst)
                    if inst.is_transpose:
                        resident.clear()
                    elif (
                        clean
                        and key is not None
                        and inst.tile_position is not None
                        and resident.get(inst.tile_position[0]) == key
                    ):
                        dropped += 1
                        continue  # redundant: same weights already loaded
                    elif key is not None and inst.tile_position is not None:
                        resident[inst.tile_position[0]] = key
                    else:
                        resident.clear()
                elif n == "InstMatmult":
                    if inst.is_transpose:
                        resident.clear()
                else:
                    pass  # non-PE instructions don't touch the array
                keep.append(inst)
            blk.instructions[:] = keep
    return dropped


def build_program():
    key = (MM_DTYPE, ISSUE, OUT_BF16, LOOP_ITERS, PROBE, LDW_OPT, DEDUPE_LDW)
    if key in _PROG_CACHE:
        return _PROG_CACHE[key]
    nc = bacc.Bacc("TRN2", target_bir_lowering=False, debug=False)
    xin_d = nc.dram_tensor(
        "xin", [NBR, 2 * CIN, HS_ROWS, WP], _mm_dt(), kind="ExternalInput"
    ).ap()
    w1x_d = nc.dram_tensor("w1x", [CIN, 9], F32, kind="ExternalInput").ap()
    w1y_d = nc.dram_tensor("w1y", [CIN, 9], F32, kind="ExternalInput").ap()
    w2xT_d = nc.dram_tensor("w2xT", [CIN, COUT], F32, kind="ExternalInput").ap()
    w2yT_d = nc.dram_tensor("w2yT", [CIN, COUT], F32, kind="ExternalInput").ap()
    out_d = nc.dram_tensor("out", [NBR, COUT, H, W], _out_dt(), kind="ExternalOutput").ap()
    with tile.TileContext(nc) as tc:
        _emit(nc, tc, xin_d, w1x_d, w1y_d, w2xT_d, w2yT_d, out_d)
    nc.compile()
    if MM_DTYPE == "bf16" and DEDUPE_LDW:
        dedupe_ldweights(nc)
    _PROG_CACHE[key] = nc
    return nc


def marshal_inputs(x, w1x, w1y, w2x, w2y):
    """Host-side data marshaling: shard over batch, zero-pad, build the
    per-partition-half copies (hsplit layout)."""
    ndt = _np_in_dt()
    x = np.asarray(x, dtype=np.float32)
    xp = np.zeros((B, NBR, CIN, HP, WP), np.float32)
    xp[:, :, :, 1 : H + 1, 1 : W + 1] = x
    xin = np.empty((B, NBR, 2 * CIN, HS_ROWS, WP), ndt)
    xin[:, :, 0:CIN] = xp[:, :, :, 0:HS_ROWS, :].astype(ndt)
    xin[:, :, CIN:] = xp[:, :, :, HP - HS_ROWS : HP, :].astype(ndt)
    w2xT = np.ascontiguousarray(np.asarray(w2x, np.float32).T)
    w2yT = np.ascontiguousarray(np.asarray(w2y, np.float32).T)
    w1x = np.ascontiguousarray(w1x, np.float32)
    w1y = np.ascontiguousarray(w1y, np.float32)
    return [
        {
            "xin": np.ascontiguousarray(xin[i]),
            "w1x": w1x,
            "w1y": w1y,
            "w2xT": w2xT,
            "w2yT": w2yT,
        }
        for i in range(B)
    ]


def kernel(x, w1x, w1y, w2x, w2y):
    global LAST_EXEC_TIME_NS, LAST_RESULTS
    nc = build_program()
    in_maps = marshal_inputs(x, w1x, w1y, w2x, w2y)
    res = bass_utils.run_bass_kernel_spmd(
        nc, in_maps, list(range(N_CORES)), trace=TRACE
    )
    LAST_EXEC_TIME_NS = res.exec_time_ns
    LAST_RESULTS = res
    out = np.stack([res.results[i]["out"] for i in range(N_CORES)], axis=0)
    return np.asarray(out, np.float32)


# revision 22
# speedup vs baseline: 2.5896x; 1.0464x over previous
"""Trainium2 Bass kernel for nn_ComplexConv2Deffangle4Dxy.

Reference math (per batch b, branch br):
    out[br] = pointwise(w2, depthwise3x3(w1, img[br]))   with zero padding P=1
      br=0 (rot): weights (w1n, w2n) where wn = (wx+wy)^2 / sum((wx+wy)^2)
      br=1 (abs): log-domain: exp(branch(log(img + EPS), w1n, w2n))
      br=2 (x):   weights (w1x, w2x)
      br=3 (y):   weights (w1y, w2y)

Kernel strategy (per NeuronCore, data-parallel over batch B=8 -> 8 cores):
  Fuse depthwise+pointwise into a single 3x3 conv whose weights are the
  outer product  Wf[o, c, k] = w2[o, c] * w1[c, k], computed as
  PSUM-accumulated matmuls over the 9 kernel offsets with
  lhsT = fused weights (K=Cin=64, M=Cout=128) and rhs = shifted image views.
  Images are zero-padded on the host (pure marshaling); for the abs branch
  Ln(x+EPS) maps the zero padding to log(EPS), matching the reference's
  pad-then-log order.  Weight normalization for the rot/abs branches is
  computed on device.

  Image layout ("hsplit"): SBUF partitions 0..63 hold padded rows 0..34,
  partitions 64..127 hold padded rows 31..65.  Lower-half output row-tiles
  read partitions 0..63 (PE row groups 0-1), upper-half tiles read
  64..127 (row groups 2-3).

  Schedule ("hb"): each branch runs as two half-branches of 4 row-tiles
  (2 lower + 2 upper, 4 PSUM banks).  Within a half-branch the issue
  order is tap-outer with lower/upper alternating, so consecutive
  matmuls land on disjoint PE row groups and execute concurrently in
  the array (HW-verified ~145ns/MM at N=512 vs 213ns serial), while the
  per-(tap, half) weight load is shared by 2 matmuls.  The previous
  half-branch's PSUM evacuation (DVE+ACT, merged into 256KB output
  DMAs) overlaps the next half-branch's matmuls.  bf16 operands give
  native split LDWEIGHTS+MATMUL (pipelined weight loads) and halve the
  DMA traffic; outputs are DMA'd as bf16 and upcast on host (total max
  rel err 4.4e-3, well inside the 2e-2 budget).

  Measured (bench2.py, short-burst slope): 128.5us (f32r baseline) ->
  49.6us.  PE floor for this fused formulation is ~31us (288 MM x 512
  cols / 2-way row-group concurrency at 2.4GHz).

  HW pitfalls hit (do not regress): concurrent K=32 same-bank
  accumulation crashes the device ("mesh desynced"); dropping the
  per-matmul redundant LDWEIGHTS makes walrus lower unpaired matmuls to
  a slower self-loading form (+4us); walrus --enable-ldw-opt fails on
  bass's pre-split LDW+MM.
"""

import sys

for _p in ("/opt/trn_rl_repo",):
    if _p not in sys.path:
        sys.path.insert(0, _p)

import ml_dtypes
import numpy as np

import concourse.bacc as bacc
import concourse.mybir as mybir
import concourse.tile as tile
from concourse import bass_utils

F32 = mybir.dt.float32
F32R = mybir.dt.float32r
BF16 = mybir.dt.bfloat16

EPS = 1e-6
N_CORES = 8
B, NBR, CIN, COUT, H, W = 8, 4, 64, 128, 64, 64
HP, WP = H + 2, W + 2          # host-padded image
HS_ROWS = 35                   # hsplit: padded rows per partition half

MM_DTYPE = "bf16"              # matmul input dtype: "f32r" | "f32" | "bf16"
OUT_BF16 = True                # DMA outputs as bf16, upcast to f32 on host
ISSUE = "hb"                   # "hb" (half-branch pipelined) | "ilv" | "seq"
# Dropping redundant same-row-group weight reloads REGRESSES on HW (53.3us
# vs 49.3us): walrus pairs each InstLdweights with its InstMatmult, and an
# unpaired matmul lowers to a slower self-loading form.  Keep False.
DEDUPE_LDW = False
LOOP_ITERS = None              # benchmarking: device-side repeat count
PROBE = ""                     # "" | "no_out" (skip evac+out-DMA) | "no_mm"
TRACE = False
LAST_EXEC_TIME_NS = None
LAST_RESULTS = None

_PROG_CACHE = {}

# walrus's LDWEIGHTS optimization: NOT needed — bass already splits bf16
# matmuls into InstLdweights + InstMatmult at the BIR level (which is what
# lets the PE reorder window pull weight loads ahead).  Enabling the walrus
# flag on the pre-split form fails codegen ("InstLdweights is not compatible
# with LDW optimization"), so keep False.
LDW_OPT = False
_orig_run_command = bass_utils.run_command


def _patched_run_command(cmd, *a, **kw):
    if (
        LDW_OPT
        and MM_DTYPE == "bf16"
        and isinstance(cmd, list)
        and "--enable-ldw-opt=false" in cmd
    ):
        cmd = ["--enable-ldw-opt=true" if c == "--enable-ldw-opt=false" else c for c in cmd]
    return _orig_run_command(cmd, *a, **kw)


bass_utils.run_command = _patched_run_command
if getattr(bass_utils, "bir_verify_and_optimise", None) is not None:
    bass_utils.bir_verify_and_optimise.__globals__["run_command"] = _patched_run_command

BRANCHES = (  # (branch index, weight set, log-domain?)
    (2, "x", False),
    (3, "y", False),
    (0, "n", False),
    (1, "n", True),
)


def _mm_dt():
    return {"f32r": F32R, "f32": F32, "bf16": BF16}[MM_DTYPE]


def _np_in_dt():
    return ml_dtypes.bfloat16 if MM_DTYPE == "bf16" else np.float32


def _out_dt():
    return BF16 if OUT_BF16 else F32


def _emit(nc, tc, xin_d, w1x_d, w1y_d, w2xT_d, w2yT_d, out_d):
    mdt = _mm_dt()
    odt = _out_dt()
    with (
        tc.tile_pool(name="wp", bufs=1) as wp,
        tc.tile_pool(name="imgp", bufs=2) as imgp,
        tc.tile_pool(name="psp", bufs=8, space="PSUM") as psp,
        tc.tile_pool(name="obp", bufs=8) as obp,
    ):
        # ---- weight prep -------------------------------------------------
        # All weight/source tiles replicated into both partition halves so
        # both PE row groups see the same fused weights.
        w1x_s = wp.tile([2 * CIN, 9], F32, tag="w1x")
        w1y_s = wp.tile([2 * CIN, 9], F32, tag="w1y")
        w2xT_s = wp.tile([2 * CIN, COUT], F32, tag="w2xT")
        w2yT_s = wp.tile([2 * CIN, COUT], F32, tag="w2yT")
        for t, d in (
            (w1x_s, w1x_d),
            (w1y_s, w1y_d),
            (w2xT_s, w2xT_d),
            (w2yT_s, w2yT_d),
        ):
            nc.sync.dma_start(out=t[0:CIN], in_=d)
            nc.sync.dma_start(out=t[CIN : 2 * CIN], in_=d)

        ones_k = wp.tile([CIN, 1], F32, tag="ones_k")
        nc.vector.memset(ones_k[:, :], 1.0)
        ones_m = wp.tile([1, 2 * CIN], F32, tag="ones_m")
        nc.vector.memset(ones_m[:, :], 1.0)
        eps_b = wp.tile([2 * CIN, 1], F32, tag="eps_b")
        nc.vector.memset(eps_b[:, :], float(EPS))
        zero_b = wp.tile([COUT, 1], F32, tag="zero_b")
        nc.vector.memset(zero_b[:, :], 0.0)

        # u1 = (w1x + w1y)^2, u2T = ((w2x + w2y)^2)^T  (both partition halves)
        u1 = wp.tile([2 * CIN, 9], F32, tag="u1")
        nc.vector.tensor_add(u1[:, :], w1x_s[:, :], w1y_s[:, :])
        nc.vector.tensor_mul(u1[:, :], u1[:, :], u1[:, :])
        u2T = wp.tile([2 * CIN, COUT], F32, tag="u2T")
        nc.vector.tensor_add(u2T[:, :], w2xT_s[:, :], w2yT_s[:, :])
        nc.vector.tensor_mul(u2T[:, :], u2T[:, :], u2T[:, :])

        # S1 = sum(u1), S2 = sum(u2) via ones-matmul + free-dim reduce
        s1v = psp.tile([1, 9], F32, tag="ps")
        nc.tensor.matmul(s1v[:, :], ones_k[:, :], u1[0:CIN, :], start=True, stop=True)
        s2v = psp.tile([1, COUT], F32, tag="ps")
        nc.tensor.matmul(s2v[:, :], ones_k[:, :], u2T[0:CIN, :], start=True, stop=True)
        s1 = wp.tile([1, 1], F32, tag="s1")
        nc.vector.tensor_reduce(
            s1[:, :], s1v[:, :], axis=mybir.AxisListType.X, op=mybir.AluOpType.add
        )
        s2 = wp.tile([1, 1], F32, tag="s2")
        nc.vector.tensor_reduce(
            s2[:, :], s2v[:, :], axis=mybir.AxisListType.X, op=mybir.AluOpType.add
        )
        inv = wp.tile([1, 1], F32, tag="inv")
        nc.vector.tensor_mul(inv[:, :], s1[:, :], s2[:, :])
        nc.vector.reciprocal(inv[:, :], inv[:, :])
        # broadcast 1/(S1*S2) to all 128 partitions
        invb_ps = psp.tile([2 * CIN, 1], F32, tag="ps")
        nc.tensor.matmul(invb_ps[:, :], ones_m[:, :], inv[:, :], start=True, stop=True)
        invb = wp.tile([2 * CIN, 1], F32, tag="invb")
        nc.vector.tensor_copy(invb[:, :], invb_ps[:, :])
        # u2T_n = u2T / (S1*S2): both normalizations in one fold
        u2Tn = wp.tile([2 * CIN, COUT], F32, tag="u2Tn")
        nc.vector.tensor_scalar(
            u2Tn[:, :], u2T[:, :], invb[:, 0:1], None, mybir.AluOpType.mult
        )

        # fused weight tiles: 9 column blocks, block k = w2T * w1[:, k],
        # identical in both partition halves (built in one op across 128
        # partitions since the scalar operand is partition-local).
        wf_tiles = {}
        for s, base, w1s in (("x", w2xT_s, w1x_s), ("y", w2yT_s, w1y_s), ("n", u2Tn, u1)):
            wf = wp.tile([2 * CIN, 9 * COUT], mdt, tag=f"wf{s}")
            for k in range(9):
                nc.vector.tensor_scalar(
                    wf[:, k * COUT : (k + 1) * COUT],
                    base[:, :],
                    w1s[:, k : k + 1],
                    None,
                    mybir.AluOpType.mult,
                )
            wf_tiles[s] = wf

        # ---- main compute ------------------------------------------------
        def emit_evac2(b, needs_log, ps2, h0, dve):
            """Evacuate 2 PSUM tiles (16 out rows) into one SBUF buffer,
            single 256KB DMA out."""
            ot = obp.tile([COUT, 2, 8, W], odt, tag="ot")
            for i in range(2):
                if needs_log:
                    nc.scalar.activation(
                        ot[:, i],
                        ps2[i][:, :, :],
                        mybir.ActivationFunctionType.Exp,
                        bias=zero_b[:, 0:1],
                    )
                elif dve:
                    nc.vector.tensor_copy(ot[:, i], ps2[i][:, :, :])
                else:
                    nc.scalar.activation(
                        ot[:, i], ps2[i][:, :, :], mybir.ActivationFunctionType.Copy
                    )
            nc.sync.dma_start(out=out_d[b, :, h0 : h0 + 16, :], in_=ot[:, :, :, :])

        def main_body():
            for b, s, needs_log in BRANCHES:
                wf = wf_tiles[s]
                img = imgp.tile([2 * CIN, HS_ROWS, WP], mdt, tag="img")
                # input DMA on the ACT HWDGE ring (qActDynamicHW) so it is
                # not queued behind the 512KB output DMAs on the SP ring
                nc.scalar.dma_start(out=img[:, :, :], in_=xin_d[b])
                if needs_log:
                    nc.scalar.activation(
                        img[:, :, :],
                        img[:, :, :],
                        mybir.ActivationFunctionType.Ln,
                        bias=eps_b[:, 0:1],
                    )
                if ISSUE == "hb":
                    # Two half-branches, each 2 lower + 2 upper row-tiles
                    # (both PE row groups stay busy); the first half-branch's
                    # evac + output DMA overlap the second's matmuls.
                    for hb in range(2):
                        ps4 = [
                            psp.tile([COUT, 8, W], F32, tag="ps", name=f"ps{t}")
                            for t in range(4)
                        ]
                        if PROBE != "no_mm":
                            _mm_hb(nc, ps4, wf, img, hb)
                        if PROBE != "no_out":
                            # ps4[0:2] = lower tiles 2hb,2hb+1 -> rows 16hb..
                            # ps4[2:4] = upper tiles -> rows 32+16hb..
                            emit_evac2(b, needs_log, ps4[0:2], 16 * hb, dve=True)
                            emit_evac2(
                                b, needs_log, ps4[2:4], 32 + 16 * hb, dve=False
                            )
                    continue
                ps = [
                    psp.tile([COUT, 8, W], F32, tag="ps", name=f"ps{t}")
                    for t in range(8)
                ]
                if PROBE != "no_mm":
                    if ISSUE == "ilv":
                        _mm_ilv(nc, ps, wf, img)
                    else:
                        _mm_seq(nc, ps, wf, img)
                if PROBE == "no_out":
                    continue
                for i in range(4):
                    emit_evac2(
                        b,
                        needs_log,
                        [ps[2 * i], ps[2 * i + 1]],
                        16 * (i % 2) + 32 * (i // 2),
                        dve=(i % 2 == 0),
                    )

        if LOOP_ITERS:
            with tc.For_i(0, LOOP_ITERS, 1):
                main_body()
        else:
            main_body()


def _rhs(img, half, tpl, k):
    """Shifted image view for out-row-tile (half, tpl) and tap k."""
    dh, dw = k // 3 - 1, k % 3 - 1
    p0, p1 = half * CIN, (half + 1) * CIN
    r = 8 * tpl + 1 + dh + half  # lower: pad row - 0; upper: pad row - 31
    c0 = 1 + dw
    return img[p0:p1, r : r + 8, c0 : c0 + W]


def _wfk(wf, k, half):
    p0, p1 = half * CIN, (half + 1) * CIN
    return wf[p0:p1, k * COUT : (k + 1) * COUT]


def _mm_ilv(nc, ps, wf, img):
    """Tap-outer, tile-inner, alternating lower/upper row groups.

    Consecutive matmuls target disjoint PE row groups (tile_position derives
    from lhsT base_partition: 0 vs 64) and distinct PSUM banks, so they run
    concurrently in the array; within a (tap, half) the 4 tiles share one
    weight load."""
    for k in range(9):
        st, sp = k == 0, k == 8
        for tpl in range(4):
            nc.tensor.matmul(
                ps[tpl][:, :, :],
                _wfk(wf, k, 0),
                _rhs(img, 0, tpl, k),
                start=st,
                stop=sp,
                skip_group_check=True,
            )
            nc.tensor.matmul(
                ps[tpl + 4][:, :, :],
                _wfk(wf, k, 1),
                _rhs(img, 1, tpl, k),
                start=st,
                stop=sp,
                skip_group_check=True,
            )


def _mm_hb(nc, ps4, wf, img, hb):
    """Half-branch hb: lower tiles {2hb, 2hb+1} -> ps4[0:2], upper tiles
    {2hb, 2hb+1} -> ps4[2:4].  Tap-outer so each (tap, half) shares one
    weight load across 2 tiles; lower/upper alternate for row-group
    concurrency.  NOTE: concurrent K=32 same-bank accumulation (4-way row
    tiling) crashes TRN2 at runtime ("mesh desynced") -- don't."""
    for k in range(9):
        st, sp = k == 0, k == 8
        for j in range(2):
            tpl = 2 * hb + j
            nc.tensor.matmul(
                ps4[j][:, :, :],
                _wfk(wf, k, 0),
                _rhs(img, 0, tpl, k),
                start=st,
                stop=sp,
                skip_group_check=True,
            )
            nc.tensor.matmul(
                ps4[2 + j][:, :, :],
                _wfk(wf, k, 1),
                _rhs(img, 1, tpl, k),
                start=st,
                stop=sp,
                skip_group_check=True,
            )


def _mm_seq(nc, ps, wf, img):
    """Baseline order: tile-outer, tap-inner (each tile's 9 taps serial)."""
    for t in range(8):
        half, tpl = (0, t) if t < 4 else (1, t - 4)
        for k in range(9):
            nc.tensor.matmul(
                ps[t][:, :, :],
                _wfk(wf, k, half),
                _rhs(img, half, tpl, k),
                start=(k == 0),
                stop=(k == 8),
                skip_group_check=True,
            )


def _ldw_key(inst):
    ap = inst.ins[0]
    bap = ap.bass_ap
    if bap is None:
        return None
    return (
        bap.tensor.name,
        bap.offset,
        tuple(tuple(p) for p in bap.ap),
        inst.tile_position,
        inst.tile_size,
        getattr(inst, "perf_mode", None),
        getattr(inst, "is_transpose", None),
    )


def dedupe_ldweights(nc):
    """Drop InstLdweights that reload the exact weights already resident in
    the same PE row group (bass emits one load per matmul even when
    consecutive same-row-group matmuls share lhsT).  Redundant loads cost
    ~53ns of serial PE time each since a row group's load cannot overlap its
    own in-flight matmul.  Only syncless duplicates are dropped, and tracking
    resets at every block boundary and at any PE instruction that could
    disturb the array (transpose-mode load, non-matmul PE op)."""
    dropped = 0
    for fn in nc.m.functions:
        for blk in fn.blocks:
            resident = {}  # tile_position[0] (row group base) -> ldw key
            keep = []
            for inst in blk.instructions:
                n = type(inst).__name__
                if n == "InstLdweights":
                    si = inst.sync_info
                    clean = si is None or (not si.on_wait and not si.on_update)
                    key = _ldw_key(inst)
                    if inst.is_transpose:
                        resident.clear()
                    elif (
                        clean
                        and key is not None
                        and inst.tile_position is not None
                        and resident.get(inst.tile_position[0]) == key
                    ):
                        dropped += 1
                        continue  # redundant: same weights already loaded
                    elif key is not None and inst.tile_position is not None:
                        resident[inst.tile_position[0]] = key
                    else:
                        resident.clear()
                elif n == "InstMatmult":
                    if inst.is_transpose:
                        resident.clear()
                else:
                    pass  # non-PE instructions don't touch the array
                keep.append(inst)
            blk.instructions[:] = keep
    return dropped


def build_program():
    key = (MM_DTYPE, ISSUE, OUT_BF16, LOOP_ITERS, PROBE, LDW_OPT, DEDUPE_LDW)
    if key in _PROG_CACHE:
        return _PROG_CACHE[key]
    nc = bacc.Bacc("TRN2", target_bir_lowering=False, debug=False)
    xin_d = nc.dram_tensor(
        "xin", [NBR, 2 * CIN, HS_ROWS, WP], _mm_dt(), kind="ExternalInput"
    ).ap()
    w1x_d = nc.dram_tensor("w1x", [CIN, 9], F32, kind="ExternalInput").ap()
    w1y_d = nc.dram_tensor("w1y", [CIN, 9], F32, kind="ExternalInput").ap()
    w2xT_d = nc.dram_tensor("w2xT", [CIN, COUT], F32, kind="ExternalInput").ap()
    w2yT_d = nc.dram_tensor("w2yT", [CIN, COUT], F32, kind="ExternalInput").ap()
    out_d = nc.dram_tensor("out", [NBR, COUT, H, W], _out_dt(), kind="ExternalOutput").ap()
    with tile.TileContext(nc) as tc:
        _emit(nc, tc, xin_d, w1x_d, w1y_d, w2xT_d, w2yT_d, out_d)
    nc.compile()
    if MM_DTYPE == "bf16" and DEDUPE_LDW:
        dedupe_ldweights(nc)
    _PROG_CACHE[key] = nc
    return nc


def marshal_inputs(x, w1x, w1y, w2x, w2y):
    """Host-side data marshaling: shard over batch, zero-pad, build the
    per-partition-half copies (hsplit layout)."""
    ndt = _np_in_dt()
    x = np.asarray(x, dtype=np.float32)
    xp = np.zeros((B, NBR, CIN, HP, WP), np.float32)
    xp[:, :, :, 1 : H + 1, 1 : W + 1] = x
    xin = np.empty((B, NBR, 2 * CIN, HS_ROWS, WP), ndt)
    xin[:, :, 0:CIN] = xp[:, :, :, 0:HS_ROWS, :].astype(ndt)
    xin[:, :, CIN:] = xp[:, :, :, HP - HS_ROWS : HP, :].astype(ndt)
    w2xT = np.ascontiguousarray(np.asarray(w2x, np.float32).T)
    w2yT = np.ascontiguousarray(np.asarray(w2y, np.float32).T)
    w1x = np.ascontiguousarray(w1x, np.float32)
    w1y = np.ascontiguousarray(w1y, np.float32)
    return [
        {
            "xin": np.ascontiguousarray(xin[i]),
            "w1x": w1x,
            "w1y": w1y,
            "w2xT": w2xT,
            "w2yT": w2yT,
        }
        for i in range(B)
    ]


def kernel(x, w1x, w1y, w2x, w2y):
    global LAST_EXEC_TIME_NS, LAST_RESULTS
    nc = build_program()
    in_maps = marshal_inputs(x, w1x, w1y, w2x, w2y)
    res = bass_utils.run_bass_kernel_spmd(
        nc, in_maps, list(range(N_CORES)), trace=TRACE
    )
    LAST_EXEC_TIME_NS = res.exec_time_ns
    LAST_RESULTS = res
    out = np.stack([res.results[i]["out"] for i in range(N_CORES)], axis=0)
    return np.asarray(out, np.float32)


# revision 30
# speedup vs baseline: 2.7046x; 1.0444x over previous
"""Trainium2 Bass kernel for nn_ComplexConv2Deffangle4Dxy.

Reference math (per batch b, branch br):
    out[br] = pointwise(w2, depthwise3x3(w1, img[br]))   with zero padding P=1
      br=0 (rot): weights (w1n, w2n) where wn = (wx+wy)^2 / sum((wx+wy)^2)
      br=1 (abs): log-domain: exp(branch(log(img + EPS), w1n, w2n))
      br=2 (x):   weights (w1x, w2x)
      br=3 (y):   weights (w1y, w2y)

Kernel strategy (per NeuronCore, data-parallel over batch B=8 -> 8 cores):
  Fuse depthwise+pointwise into a single 3x3 conv whose weights are the
  outer product  Wf[o, c, k] = w2[o, c] * w1[c, k], computed as
  PSUM-accumulated matmuls over the 9 kernel offsets with
  lhsT = fused weights (K=Cin=64, M=Cout=128) and rhs = shifted image views.
  Images are zero-padded on the host (pure marshaling); for the abs branch
  Ln(x+EPS) maps the zero padding to log(EPS), matching the reference's
  pad-then-log order.  Weight normalization for the rot/abs branches is
  computed on device.

  Image layout ("hsplit"): SBUF partitions 0..63 hold padded rows 0..34,
  partitions 64..127 hold padded rows 31..65.  Lower-half output row-tiles
  read partitions 0..63 (PE row groups 0-1), upper-half tiles read
  64..127 (row groups 2-3).

  Schedule ("hb"): each branch runs as two half-branches of 4 row-tiles
  (2 lower + 2 upper, 4 PSUM banks).  Within a half-branch the issue
  order is tap-outer with lower/upper alternating, so consecutive
  matmuls land on disjoint PE row groups and execute concurrently in
  the array (HW-verified ~145ns/MM at N=512 vs 213ns serial), while the
  per-(tap, half) weight load is shared by 2 matmuls.  The previous
  half-branch's PSUM evacuation (DVE+ACT, merged into 256KB output
  DMAs) overlaps the next half-branch's matmuls.  bf16 operands give
  native split LDWEIGHTS+MATMUL (pipelined weight loads) and halve the
  DMA traffic; outputs are DMA'd as bf16 and upcast on host (total max
  rel err 4.4e-3, well inside the 2e-2 budget).

  Measured (bench2.py, short-burst slope): 128.5us (f32r baseline) ->
  49.6us.  PE floor for this fused formulation is ~31us (288 MM x 512
  cols / 2-way row-group concurrency at 2.4GHz).

  HW pitfalls hit (do not regress): concurrent K=32 same-bank
  accumulation crashes the device ("mesh desynced"); dropping the
  per-matmul redundant LDWEIGHTS makes walrus lower unpaired matmuls to
  a slower self-loading form (+4us); walrus --enable-ldw-opt fails on
  bass's pre-split LDW+MM.
"""

import sys

for _p in ("/opt/trn_rl_repo",):
    if _p not in sys.path:
        sys.path.insert(0, _p)

import ml_dtypes
import numpy as np

import concourse.bacc as bacc
import concourse.mybir as mybir
import concourse.tile as tile
from concourse import bass_utils

F32 = mybir.dt.float32
F32R = mybir.dt.float32r
BF16 = mybir.dt.bfloat16

EPS = 1e-6
N_CORES = 8
B, NBR, CIN, COUT, H, W = 8, 4, 64, 128, 64, 64
HP, WP = H + 2, W + 2          # host-padded image
HS_ROWS = 35                   # hsplit: padded rows per partition half

MM_DTYPE = "bf16"              # matmul input dtype: "f32r" | "f32" | "bf16"
OUT_BF16 = True                # DMA outputs as bf16, upcast to f32 on host
ISSUE = "hb"                   # "hb" (half-branch pipelined) | "ilv" | "seq"
# Dropping redundant same-row-group weight reloads REGRESSES on HW (53.3us
# vs 49.3us): walrus pairs each InstLdweights with its InstMatmult, and an
# unpaired matmul lowers to a slower self-loading form.  Keep False.
DEDUPE_LDW = False
STRIP_MM_SEMS = False          # keep PE sem incs only on group-final matmuls
LOOP_ITERS = None              # benchmarking: device-side repeat count
PROBE = ""                     # "" | "no_out" (skip evac+out-DMA) | "no_mm"
TRACE = False
LAST_EXEC_TIME_NS = None
LAST_RESULTS = None

_PROG_CACHE = {}

# walrus's LDWEIGHTS optimization: NOT needed — bass already splits bf16
# matmuls into InstLdweights + InstMatmult at the BIR level (which is what
# lets the PE reorder window pull weight loads ahead).  Enabling the walrus
# flag on the pre-split form fails codegen ("InstLdweights is not compatible
# with LDW optimization"), so keep False.
LDW_OPT = False
_orig_run_command = bass_utils.run_command


def _patched_run_command(cmd, *a, **kw):
    if (
        LDW_OPT
        and MM_DTYPE == "bf16"
        and isinstance(cmd, list)
        and "--enable-ldw-opt=false" in cmd
    ):
        cmd = ["--enable-ldw-opt=true" if c == "--enable-ldw-opt=false" else c for c in cmd]
    return _orig_run_command(cmd, *a, **kw)


bass_utils.run_command = _patched_run_command
if getattr(bass_utils, "bir_verify_and_optimise", None) is not None:
    bass_utils.bir_verify_and_optimise.__globals__["run_command"] = _patched_run_command

BRANCHES = (  # (branch index, weight set, log-domain?)
    (2, "x", False),
    (3, "y", False),
    (0, "n", False),
    (1, "n", True),
)


def _mm_dt():
    return {"f32r": F32R, "f32": F32, "bf16": BF16}[MM_DTYPE]


def _np_in_dt():
    return ml_dtypes.bfloat16 if MM_DTYPE == "bf16" else np.float32


def _out_dt():
    return BF16 if OUT_BF16 else F32


def _emit(nc, tc, xin_d, w1x_d, w1y_d, w2xT_d, w2yT_d, out_d):
    mdt = _mm_dt()
    odt = _out_dt()
    with (
        tc.tile_pool(name="wp", bufs=1) as wp,
        tc.tile_pool(name="imgp", bufs=3) as imgp,
        tc.tile_pool(name="psp", bufs=4, space="PSUM") as psp,
        tc.tile_pool(name="obp", bufs=8) as obp,
    ):
        # ---- weight prep -------------------------------------------------
        # All weight/source tiles replicated into both partition halves so
        # both PE row groups see the same fused weights.
        w1x_s = wp.tile([2 * CIN, 9], F32, tag="w1x")
        w1y_s = wp.tile([2 * CIN, 9], F32, tag="w1y")
        w2xT_s = wp.tile([2 * CIN, COUT], F32, tag="w2xT")
        w2yT_s = wp.tile([2 * CIN, COUT], F32, tag="w2yT")
        for t, d in (
            (w1x_s, w1x_d),
            (w1y_s, w1y_d),
            (w2xT_s, w2xT_d),
            (w2yT_s, w2yT_d),
        ):
            nc.sync.dma_start(out=t[0:CIN], in_=d)
            nc.sync.dma_start(out=t[CIN : 2 * CIN], in_=d)

        ones_k = wp.tile([CIN, 1], F32, tag="ones_k")
        nc.vector.memset(ones_k[:, :], 1.0)
        ones_m = wp.tile([1, 2 * CIN], F32, tag="ones_m")
        nc.vector.memset(ones_m[:, :], 1.0)
        eps_b = wp.tile([2 * CIN, 1], F32, tag="eps_b")
        nc.vector.memset(eps_b[:, :], float(EPS))
        zero_b = wp.tile([COUT, 1], F32, tag="zero_b")
        nc.vector.memset(zero_b[:, :], 0.0)

        # u1 = (w1x + w1y)^2, u2T = ((w2x + w2y)^2)^T  (both partition halves)
        u1 = wp.tile([2 * CIN, 9], F32, tag="u1")
        nc.vector.tensor_add(u1[:, :], w1x_s[:, :], w1y_s[:, :])
        nc.vector.tensor_mul(u1[:, :], u1[:, :], u1[:, :])
        u2T = wp.tile([2 * CIN, COUT], F32, tag="u2T")
        nc.vector.tensor_add(u2T[:, :], w2xT_s[:, :], w2yT_s[:, :])
        nc.vector.tensor_mul(u2T[:, :], u2T[:, :], u2T[:, :])

        # S1 = sum(u1), S2 = sum(u2) via ones-matmul + free-dim reduce
        s1v = psp.tile([1, 9], F32, tag="ps", bufs=4)
        nc.tensor.matmul(s1v[:, :], ones_k[:, :], u1[0:CIN, :], start=True, stop=True)
        s2v = psp.tile([1, COUT], F32, tag="ps", bufs=4)
        nc.tensor.matmul(s2v[:, :], ones_k[:, :], u2T[0:CIN, :], start=True, stop=True)
        s1 = wp.tile([1, 1], F32, tag="s1")
        nc.vector.tensor_reduce(
            s1[:, :], s1v[:, :], axis=mybir.AxisListType.X, op=mybir.AluOpType.add
        )
        s2 = wp.tile([1, 1], F32, tag="s2")
        nc.vector.tensor_reduce(
            s2[:, :], s2v[:, :], axis=mybir.AxisListType.X, op=mybir.AluOpType.add
        )
        inv = wp.tile([1, 1], F32, tag="inv")
        nc.vector.tensor_mul(inv[:, :], s1[:, :], s2[:, :])
        nc.vector.reciprocal(inv[:, :], inv[:, :])
        # broadcast 1/(S1*S2) to all 128 partitions
        invb_ps = psp.tile([2 * CIN, 1], F32, tag="ps", bufs=4)
        nc.tensor.matmul(invb_ps[:, :], ones_m[:, :], inv[:, :], start=True, stop=True)
        invb = wp.tile([2 * CIN, 1], F32, tag="invb")
        nc.vector.tensor_copy(invb[:, :], invb_ps[:, :])
        # u2T_n = u2T / (S1*S2): both normalizations in one fold
        u2Tn = wp.tile([2 * CIN, COUT], F32, tag="u2Tn")
        nc.vector.tensor_scalar(
            u2Tn[:, :], u2T[:, :], invb[:, 0:1], None, mybir.AluOpType.mult
        )

        # fused weight tiles: 9 column blocks, block k = w2T * w1[:, k],
        # identical in both partition halves (built in one op across 128
        # partitions since the scalar operand is partition-local).
        wf_tiles = {}
        for s, base, w1s in (("x", w2xT_s, w1x_s), ("y", w2yT_s, w1y_s), ("n", u2Tn, u1)):
            wf = wp.tile([2 * CIN, 9 * COUT], mdt, tag=f"wf{s}")
            for k in range(9):
                nc.vector.tensor_scalar(
                    wf[:, k * COUT : (k + 1) * COUT],
                    base[:, :],
                    w1s[:, k : k + 1],
                    None,
                    mybir.AluOpType.mult,
                )
            wf_tiles[s] = wf

        # ---- main compute ------------------------------------------------
        def emit_evac_wide(b, needs_log, psw, h0, dve):
            """Evacuate one 2-bank PSUM tile (16 out rows) in a single wide
            engine op, then one 256KB DMA out."""
            ot = obp.tile([COUT, 2, 8, W], odt, tag="ot")
            if needs_log:
                nc.scalar.activation(
                    ot[:, :, :, :],
                    psw[:, :, :, :],
                    mybir.ActivationFunctionType.Exp,
                    bias=zero_b[:, 0:1],
                )
            elif dve:
                nc.vector.tensor_copy(ot[:, :, :, :], psw[:, :, :, :])
            else:
                nc.scalar.activation(
                    ot[:, :, :, :], psw[:, :, :, :], mybir.ActivationFunctionType.Copy
                )
            if PROBE != "no_dma":
                nc.sync.dma_start(out=out_d[b, :, h0 : h0 + 16, :], in_=ot[:, :, :, :])

        def main_body():
            for b, s, needs_log in BRANCHES:
                wf = wf_tiles[s]
                img = imgp.tile([2 * CIN, HS_ROWS, WP], mdt, tag="img")
                # input DMA on the ACT HWDGE ring (qActDynamicHW) so it is
                # not queued behind the 512KB output DMAs on the SP ring
                nc.scalar.dma_start(out=img[:, :, :], in_=xin_d[b])
                if needs_log:
                    nc.scalar.activation(
                        img[:, :, :],
                        img[:, :, :],
                        mybir.ActivationFunctionType.Ln,
                        bias=eps_b[:, 0:1],
                    )
                if ISSUE == "hb":
                    # Two half-branches, each 2 lower + 2 upper row-tiles
                    # (both PE row groups stay busy); the first half-branch's
                    # evac + output DMA overlap the second's matmuls.  Each
                    # pair of tiles accumulates into one 2-bank PSUM tile
                    # ([COUT, 2, 8, W]): each matmul output slice [:, j] is
                    # exactly one bank, and the pair evacuates in one wide op.
                    for hb in range(2):
                        ps_lo = psp.tile(
                            [COUT, 2, 8, W], F32, tag="ps", bufs=4, name="pslo"
                        )
                        ps_hi = psp.tile(
                            [COUT, 2, 8, W], F32, tag="ps", bufs=4, name="pshi"
                        )
                        if PROBE != "no_mm":
                            _mm_hb(
                                nc,
                                [ps_lo[:, 0], ps_lo[:, 1], ps_hi[:, 0], ps_hi[:, 1]],
                                wf,
                                img,
                                hb,
                            )
                        if PROBE != "no_out":
                            # ps_lo = lower tiles 2hb,2hb+1 -> rows 16hb..
                            # ps_hi = upper tiles -> rows 32+16hb..
                            emit_evac_wide(b, needs_log, ps_lo, 16 * hb, dve=True)
                            emit_evac_wide(
                                b, needs_log, ps_hi, 32 + 16 * hb, dve=False
                            )
                    continue
                psw = [
                    psp.tile([COUT, 2, 8, W], F32, tag="ps", bufs=4, name=f"psw{i}")
                    for i in range(4)
                ]
                ps = [psw[i // 2][:, i % 2] for i in range(8)]
                if PROBE != "no_mm":
                    if ISSUE == "ilv":
                        _mm_ilv(nc, ps, wf, img)
                    else:
                        _mm_seq(nc, ps, wf, img)
                if PROBE == "no_out":
                    continue
                for i in range(4):
                    emit_evac_wide(
                        b,
                        needs_log,
                        psw[i],
                        16 * (i % 2) + 32 * (i // 2),
                        dve=(i % 2 == 0),
                    )

        if LOOP_ITERS:
            with tc.For_i(0, LOOP_ITERS, 1):
                main_body()
        else:
            main_body()


def _rhs(img, half, tpl, k):
    """Shifted image view for out-row-tile (half, tpl) and tap k."""
    dh, dw = k // 3 - 1, k % 3 - 1
    p0, p1 = half * CIN, (half + 1) * CIN
    r = 8 * tpl + 1 + dh + half  # lower: pad row - 0; upper: pad row - 31
    c0 = 1 + dw
    return img[p0:p1, r : r + 8, c0 : c0 + W]


def _wfk(wf, k, half):
    p0, p1 = half * CIN, (half + 1) * CIN
    return wf[p0:p1, k * COUT : (k + 1) * COUT]


def _mm_ilv(nc, ps, wf, img):
    """Tap-outer, tile-inner, alternating lower/upper row groups.

    Consecutive matmuls target disjoint PE row groups (tile_position derives
    from lhsT base_partition: 0 vs 64) and distinct PSUM banks, so they run
    concurrently in the array; within a (tap, half) the 4 tiles share one
    weight load."""
    for k in range(9):
        st, sp = k == 0, k == 8
        for tpl in range(4):
            nc.tensor.matmul(
                ps[tpl][:, :, :],
                _wfk(wf, k, 0),
                _rhs(img, 0, tpl, k),
                start=st,
                stop=sp,
                skip_group_check=True,
            )
            nc.tensor.matmul(
                ps[tpl + 4][:, :, :],
                _wfk(wf, k, 1),
                _rhs(img, 1, tpl, k),
                start=st,
                stop=sp,
                skip_group_check=True,
            )


def _mm_hb(nc, ps4, wf, img, hb):
    """Half-branch hb: lower tiles {2hb, 2hb+1} -> ps4[0:2], upper tiles
    {2hb, 2hb+1} -> ps4[2:4].  Tap-outer so each (tap, half) shares one
    weight load across 2 tiles; lower/upper alternate for row-group
    concurrency.  NOTE: concurrent K=32 same-bank accumulation (4-way row
    tiling) crashes TRN2 at runtime ("mesh desynced") -- don't."""
    for k in range(9):
        st, sp = k == 0, k == 8
        for j in range(2):
            tpl = 2 * hb + j
            nc.tensor.matmul(
                ps4[j][:, :, :],
                _wfk(wf, k, 0),
                _rhs(img, 0, tpl, k),
                start=st,
                stop=sp,
                skip_group_check=True,
            )
            nc.tensor.matmul(
                ps4[2 + j][:, :, :],
                _wfk(wf, k, 1),
                _rhs(img, 1, tpl, k),
                start=st,
                stop=sp,
                skip_group_check=True,
            )


def _mm_seq(nc, ps, wf, img):
    """Baseline order: tile-outer, tap-inner (each tile's 9 taps serial)."""
    for t in range(8):
        half, tpl = (0, t) if t < 4 else (1, t - 4)
        for k in range(9):
            nc.tensor.matmul(
                ps[t][:, :, :],
                _wfk(wf, k, half),
                _rhs(img, half, tpl, k),
                start=(k == 0),
                stop=(k == 8),
                skip_group_check=True,
            )


def _ldw_key(inst):
    ap = inst.ins[0]
    bap = ap.bass_ap
    if bap is None:
        return None
    return (
        bap.tensor.name,
        bap.offset,
        tuple(tuple(p) for p in bap.ap),
        inst.tile_position,
        inst.tile_size,
        getattr(inst, "perf_mode", None),
        getattr(inst, "is_transpose", None),
    )


def dedupe_ldweights(nc):
    """Drop InstLdweights that reload the exact weights already resident in
    the same PE row group (bass emits one load per matmul even when
    consecutive same-row-group matmuls share lhsT).  Redundant loads cost
    ~53ns of serial PE time each since a row group's load cannot overlap its
    own in-flight matmul.  Only syncless duplicates are dropped, and tracking
    resets at every block boundary and at any PE instruction that could
    disturb the array (transpose-mode load, non-matmul PE op)."""
    dropped = 0
    for fn in nc.m.functions:
        for blk in fn.blocks:
            resident = {}  # tile_position[0] (row group base) -> ldw key
            keep = []
            for inst in blk.instructions:
                n = type(inst).__name__
                if n == "InstLdweights":
                    si = inst.sync_info
                    clean = si is None or (not si.on_wait and not si.on_update)
                    key = _ldw_key(inst)
                    if inst.is_transpose:
                        resident.clear()
                    elif (
                        clean
                        and key is not None
                        and inst.tile_position is not None
                        and resident.get(inst.tile_position[0]) == key
                    ):
                        dropped += 1
                        continue  # redundant: same weights already loaded
                    elif key is not None and inst.tile_position is not None:
                        resident[inst.tile_position[0]] = key
                    else:
                        resident.clear()
                elif n == "InstMatmult":
                    if inst.is_transpose:
                        resident.clear()
                else:
                    pass  # non-PE instructions don't touch the array
                keep.append(inst)
            blk.instructions[:] = keep
    return dropped


def strip_mm_sem_incs(nc):
    """Keep the PE semaphore increment only on group-final matmuls
    (stop_tensor_calc=True); rewrite every wait threshold on that semaphore
    to count kept increments.  Tile emits one inc per matmul for dependency
    tracking; the EVT_SEM register writes serialize on the PE NX (~26ns
    each), so 288 per iteration is ~7.5us of potential issue overhead.

    Consumers' thresholds land on group-final or branch-final matmuls (PSUM
    read-after-write and image WAR edges), whose incs are kept, so the
    rewrite is exact; a threshold landing on a stripped inc rounds UP to the
    next kept one (later than needed: safe, never earlier)."""
    import concourse.mybir as mb

    pe_engine = mb.EngineType.PE
    stripped = 0
    for fn in nc.m.functions:
        for blk in fn.blocks:
            # collect matmul incs per PE sem, in scheduled order
            seq = {}  # sem -> list of (inst, update, kept?)
            for inst in blk.instructions:
                if type(inst).__name__ != "InstMatmult" or not inst.sync_info:
                    continue
                for u in inst.sync_info.on_update:
                    if u.update_mode == "sem-inc":
                        seq.setdefault(u.ant_name, []).append(
                            (inst, u, bool(inst.stop_tensor_calc))
                        )
            for sem, entries in seq.items():
                if all(k for _, _, k in entries):
                    continue
                # cum ordinal at entry i (1-based); transfer preserves cum at
                # kept entries, so newv maps: kept ordinal -> itself;
                # stripped ordinal -> prev kept (PE-self waits, throttles) or
                # next kept (cross-engine waits, real dependencies)
                cum = []
                c = 0
                for _, u, _ in entries:
                    c += u.update_value
                    cum.append(c)
                kept_flags = [k for _, _, k in entries]
                prev_kept = {}
                next_kept = {}
                last = 0
                for i, k in enumerate(kept_flags):
                    if k:
                        last = cum[i]
                    prev_kept[cum[i]] = last
                nxt = cum[-1]
                for i in reversed(range(len(kept_flags))):
                    if kept_flags[i]:
                        nxt = cum[i]
                    next_kept[cum[i]] = nxt
                maxc = cum[-1]
                for inst in blk.instructions:
                    si = inst.sync_info
                    if si is None:
                        continue
                    for w in si.on_wait:
                        if (
                            getattr(w, "ant_name", None) == sem
                            and w.wait_mode == "sem-ge-imm"
                            and w.wait_value in prev_kept
                        ):
                            v = w.wait_value
                            if inst.engine == pe_engine:
                                w.wait_value = prev_kept[v]
                            else:
                                w.wait_value = next_kept[v]
                # transfer stripped inc values to the next kept inc so
                # cumulative counts at kept points stay exact
                pending = 0
                for inst, u, k in entries:
                    if k:
                        u.update_value += pending
                        pending = 0
                    else:
                        pending += u.update_value
                        inst.sync_info.on_update = [
                            x for x in inst.sync_info.on_update if x is not u
                        ]
                        stripped += 1
                if pending:
                    # trailing stripped incs: restore the last one to flush
                    raise RuntimeError(
                        f"strip_mm_sem_incs: trailing pending {pending} on {sem}"
                    )
    return stripped


def build_program():
    key = (
        MM_DTYPE, ISSUE, OUT_BF16, LOOP_ITERS, PROBE, LDW_OPT, DEDUPE_LDW,
        STRIP_MM_SEMS,
    )
    if key in _PROG_CACHE:
        return _PROG_CACHE[key]
    nc = bacc.Bacc("TRN2", target_bir_lowering=False, debug=False)
    xin_d = nc.dram_tensor(
        "xin", [NBR, 2 * CIN, HS_ROWS, WP], _mm_dt(), kind="ExternalInput"
    ).ap()
    w1x_d = nc.dram_tensor("w1x", [CIN, 9], F32, kind="ExternalInput").ap()
    w1y_d = nc.dram_tensor("w1y", [CIN, 9], F32, kind="ExternalInput").ap()
    w2xT_d = nc.dram_tensor("w2xT", [CIN, COUT], F32, kind="ExternalInput").ap()
    w2yT_d = nc.dram_tensor("w2yT", [CIN, COUT], F32, kind="ExternalInput").ap()
    out_d = nc.dram_tensor("out", [NBR, COUT, H, W], _out_dt(), kind="ExternalOutput").ap()
    with tile.TileContext(nc) as tc:
        _emit(nc, tc, xin_d, w1x_d, w1y_d, w2xT_d, w2yT_d, out_d)
    nc.compile()
    if MM_DTYPE == "bf16" and DEDUPE_LDW:
        dedupe_ldweights(nc)
    if STRIP_MM_SEMS:
        strip_mm_sem_incs(nc)
    _PROG_CACHE[key] = nc
    return nc


def marshal_inputs(x, w1x, w1y, w2x, w2y):
    """Host-side data marshaling: shard over batch, zero-pad, build the
    per-partition-half copies (hsplit layout)."""
    ndt = _np_in_dt()
    x = np.asarray(x, dtype=np.float32)
    xp = np.zeros((B, NBR, CIN, HP, WP), np.float32)
    xp[:, :, :, 1 : H + 1, 1 : W + 1] = x
    xin = np.empty((B, NBR, 2 * CIN, HS_ROWS, WP), ndt)
    xin[:, :, 0:CIN] = xp[:, :, :, 0:HS_ROWS, :].astype(ndt)
    xin[:, :, CIN:] = xp[:, :, :, HP - HS_ROWS : HP, :].astype(ndt)
    w2xT = np.ascontiguousarray(np.asarray(w2x, np.float32).T)
    w2yT = np.ascontiguousarray(np.asarray(w2y, np.float32).T)
    w1x = np.ascontiguousarray(w1x, np.float32)
    w1y = np.ascontiguousarray(w1y, np.float32)
    return [
        {
            "xin": np.ascontiguousarray(xin[i]),
            "w1x": w1x,
            "w1y": w1y,
            "w2xT": w2xT,
            "w2yT": w2yT,
        }
        for i in range(B)
    ]


def kernel(x, w1x, w1y, w2x, w2y):
    global LAST_EXEC_TIME_NS, LAST_RESULTS
    nc = build_program()
    in_maps = marshal_inputs(x, w1x, w1y, w2x, w2y)
    res = bass_utils.run_bass_kernel_spmd(
        nc, in_maps, list(range(N_CORES)), trace=TRACE
    )
    LAST_EXEC_TIME_NS = res.exec_time_ns
    LAST_RESULTS = res
    out = np.stack([res.results[i]["out"] for i in range(N_CORES)], axis=0)
    return np.asarray(out, np.float32)
